# revision 36
# baseline (speedup 1.0000x reference)
"""Distance-correlation (DcorLoss) kernel for 8 trn2 NeuronCores.

Math: for x, y [n=8192, d=128]:
  a = pairwise_dist(x), b = pairwise_dist(y)   (n x n, symmetric, zero diag)
  A = double_center(a), B = double_center(b)
  dcor = -sqrt(sum(A*B)) / sqrt(sqrt(sum(A*A)) * sqrt(sum(B*B)))

Identities (never materialize A/B): with at = a - mu,
  sum(A o B) = sum(at o bt) - 2/n dot(rs_at, rs_bt) + sum(at)sum(bt)/n^2
and sum_ij a_ij^2 has a closed form from norms + column sums (host-only).
So the device only streams row sums of a/b and sum (a-mu)*b.

SYMMETRY: a is symmetric, so each unordered block-pair is visited once.
Core c's columns are rotated by c*1024; it processes local windows 0..4
(blocks c..c+4). Windows 1..4 also emit per-column sums (PE ones-matmul
chains into PSUM, DMA'd out raw) which the host mirrors into the row sums
of the partner blocks. Window 4 is covered by both cores of the pair
{c, c+4}; the host halves those contributions. 40 tiles/core vs 64.

Device work per (128-row x 1024-col) tile:
  PE:   psum = -2*x_blk^T x (K=128 bf16) + onesK (x) [n_hi;n_lo;0...] (K=128,
        keeps the HAM clock-gate at 8/8) (+ mu^2*I on the diag window 0)
  ACT:  t = sqrt(psum + n_i) [fp32 bias], accum_out -> row sums
  PE:   column sums of t_a/t_b (deferred one tile to avoid PE stalls)
  DVE:  (t_a - mu) * t_b -> accum_out
All operands precomputed on HOST (bf16 + f32 norms); host combines in fp64.
"""

import numpy as np
import ml_dtypes

import concourse.bass as bass
import concourse.tile as tile
from concourse import bacc, mybir
from concourse.bass_utils import run_bass_kernel_spmd

P = 128            # partitions / d
N = 8192           # points
NCORES = 8
BLK = N // NCORES  # 1024 rows per core
CI_N = BLK // P    # 8 row chunks per core
W = 1024           # column window
NW = 5             # local windows per core (blocks c..c+4)
MU = 16.0          # ~E[pairwise dist] for randn d=128; any constant is exact
MU2 = MU * MU
ST_W = NW * CI_N   # 40 accum columns
BF = ml_dtypes.bfloat16

_programs = {}


def _build():
    dt = mybir.dt
    f32 = dt.float32
    bf16 = dt.bfloat16
    A = mybir.AluOpType
    AF = mybir.ActivationFunctionType

    nc = bacc.Bacc("TRN2", target_bir_lowering=False, debug=False,
                   num_devices=NCORES)

    NC = NW * W  # 5120 columns resident per core
    dxT = nc.dram_tensor("xT", [P, NC], bf16, kind="ExternalInput").ap()
    dyT = nc.dram_tensor("yT", [P, NC], bf16, kind="ExternalInput").ap()
    dxb = nc.dram_tensor("xblkT", [P, BLK], bf16, kind="ExternalInput").ap()
    dyb = nc.dram_tensor("yblkT", [P, BLK], bf16, kind="ExternalInput").ap()
    # norm rows zero-padded to 128 partitions: K=128 norm-add matmuls keep
    # HAM array activity high enough to hold the 8/8 clock state
    dnfx = nc.dram_tensor("nfx", [P, NC], bf16, kind="ExternalInput").ap()
    dnfy = nc.dram_tensor("nfy", [P, NC], bf16, kind="ExternalInput").ap()
    dnbx = nc.dram_tensor("nbx", [P, CI_N], f32, kind="ExternalInput").ap()
    dnby = nc.dram_tensor("nby", [P, CI_N], f32, kind="ExternalInput").ap()
    ddg = nc.dram_tensor("diagm", [P, P], bf16, kind="ExternalInput").ap()
    dew = nc.dram_tensor("eyewide", [P, 4 * 512], bf16,
                         kind="ExternalInput").ap()
    dout = nc.dram_tensor("out", [P, 3 * ST_W], f32,
                          kind="ExternalOutput").ap()
    # col sums: per window w=1..4: [a_h0, a_h1, b_h0, b_h1] x 512
    dcol = nc.dram_tensor("colsums", [1, 4 * 4 * 512], f32,
                          kind="ExternalOutput").ap()

    with tile.TileContext(nc) as tc:
        with tc.tile_pool(name="const", bufs=1) as cp, \
             tc.tile_pool(name="psum", bufs=1, space="PSUM") as pp, \
             tc.tile_pool(name="ab", bufs=3) as abp, \
             tc.tile_pool(name="trd", bufs=2) as trd:

            xTc = cp.tile([P, NC], bf16, tag="xTc")
            yTc = cp.tile([P, NC], bf16, tag="yTc")
            xblk2 = cp.tile([P, BLK], bf16, tag="xblk2")
            yblk2 = cp.tile([P, BLK], bf16, tag="yblk2")
            nfhl_x = cp.tile([P, NC], bf16, tag="nfhl_x")
            nfhl_y = cp.tile([P, NC], bf16, tag="nfhl_y")
            nbx = cp.tile([P, CI_N], f32, tag="nbx")
            nby = cp.tile([P, CI_N], f32, tag="nby")
            diag_m = cp.tile([P, P], bf16, tag="diag_m")
            eyew_m = cp.tile([P, 4 * 512], bf16, tag="eyew_m")
            onesK = cp.tile([P, P], bf16, tag="onesK")
            nc.vector.memset(onesK[:], 1.0)

            # windows processed [1,2,3,4,0]: the diag window (no colsum
            # chain) goes last so the final chain closes inside it
            WORDER = [1, 2, 3, 4, 0]

            # input DMAs: the warm-up's private copy of xblk2[:, :256]
            # first (small, lands in ~3us), then first-used-first in
            # 512-col chunks so the first mains can start ~7us
            wu = cp.tile([P, 256], bf16, tag="wu")
            nc.sync.dma_start(wu[:], dxb[:, 0:256])
            nc.sync.dma_start(xblk2[:], dxb[:])
            nc.sync.dma_start(yblk2[:], dyb[:])
            nc.sync.dma_start(nbx[:], dnbx[:])
            nc.sync.dma_start(nby[:], dnby[:])
            nc.sync.dma_start(diag_m[:], ddg[:])
            nc.sync.dma_start(eyew_m[:], dew[:])
            # window 1 (processed first) in fine 512-col chunks; the rest
            # as two big transfers per tensor — fewer dma_start issues
            # (each costs ~0.6us of sequencer time, serialized)
            for h2 in range(2):
                sl = bass.ds(1 * W + h2 * 512, 512)
                nc.sync.dma_start(xTc[:, sl], dxT[:, sl])
                nc.sync.dma_start(yTc[:, sl], dyT[:, sl])
                nc.sync.dma_start(nfhl_x[:, sl], dnfx[:, sl])
                nc.sync.dma_start(nfhl_y[:, sl], dnfy[:, sl])
            rest = bass.ds(2 * W, 3 * W)     # windows 2,3,4 (contiguous)
            w0 = bass.ds(0, W)               # window 0 (processed last)
            for t_, d_ in ((xTc, dxT), (yTc, dyT),
                           (nfhl_x, dnfx), (nfhl_y, dnfy)):
                nc.sync.dma_start(t_[:, rest], d_[:, rest])
                nc.sync.dma_start(t_[:, w0], d_[:, w0])

            # PE warm-up: HAM un-throttles (half clock -> full) only after
            # ~3.4us of sustained K=128 array activity
            wt = pp.tile([P, W], f32, tag="ps", bufs=3)
            for _ in range(18):
                nc.tensor.matmul(wt[:, 0:256], wu[:, 0:P], wu[:],
                                 start=True, stop=True)

            st = [cp.tile([P, ST_W], f32, tag=f"st{q}", name=f"st{q}")
                  for q in range(3)]
            colstage = cp.tile([1, 4 * 4 * 512], f32, tag="colstage")

            # ── main loop (w outer for colsum chains) ─────────────────
            pend = []          # deferred colsum matmuls from previous tile
            colt = {}          # window -> 4 psum colsum tiles
            _shipped = set()   # (q, w) st regions already DMA'd in-loop
            for idx, w in enumerate(WORDER):
                if idx >= 1:
                    # ship the previous window's finished st columns now
                    pw_ = WORDER[idx - 1]
                    for q in range(3):
                        nc.sync.dma_start(
                            dout[:, bass.ds(q * ST_W + pw_ * CI_N, CI_N)],
                            st[q][:, bass.ts(pw_, CI_N)])
                        _shipped.add((q, pw_))
                for ci in range(CI_N):
                    col = w * CI_N + ci
                    h_diag = ci // 4
                    psA = pp.tile([P, W], f32, tag="ps", bufs=3)
                    psB = pp.tile([P, W], f32, tag="ps", bufs=3)
                    for ps_, blk2, full in ((psA, xblk2, xTc),
                                            (psB, yblk2, yTc)):
                        for h in range(2):
                            nc.tensor.matmul(
                                ps_[:, bass.ds(h * 512, 512)],
                                blk2[:, bass.ts(ci, P)],
                                full[:, bass.ds(w * W + h * 512, 512)],
                                start=True, stop=False)
                    for ps_, nfhl in ((psA, nfhl_x), (psB, nfhl_y)):
                        for h in range(2):
                            nc.tensor.matmul(
                                ps_[:, bass.ds(h * 512, 512)], onesK[:],
                                nfhl[:, bass.ds(w * W + h * 512, 512)],
                                start=False,
                                stop=(w != 0 or h != h_diag))
                    if w == 0:
                        for ps_ in (psA, psB):
                            nc.tensor.matmul(
                                ps_[:, bass.ds(h_diag * 512, 512)],
                                diag_m[:, 0:P],
                                eyew_m[:, bass.ts(ci % 4, 512)],
                                start=False, stop=True)

                    # older tiles' colsum matmuls go here: their t_a/t_b
                    # (and any colsum-bank drains) are long since done, so
                    # the PE never stalls. 2-deep deferral gives window-
                    # boundary drains a full tile of slack.
                    while len(pend) >= 2:
                        pend.pop(0)()

                    aT = abp.tile([P, W], f32, tag="a")
                    bT = abp.tile([P, W], f32, tag="b")
                    nc.scalar.activation(aT[:], psA[:], AF.Sqrt,
                                         bias=nbx[:, ci:ci + 1],
                                         accum_out=st[0][:, col:col + 1])
                    nc.scalar.activation(bT[:], psB[:], AF.Sqrt,
                                         bias=nby[:, ci:ci + 1],
                                         accum_out=st[1][:, col:col + 1])
                    t0 = trd.tile([P, W], bf16, tag="t")
                    nc.vector.scalar_tensor_tensor(
                        t0[:], aT[:], MU, bT[:], op0=A.subtract, op1=A.mult,
                        accum_out=st[2][:, col:col + 1])

                    if w >= 1:
                        # bf16 shadows of a/b, only for the colsum matmuls
                        # (row sums and products stay fp32-clean)
                        ac = abp.tile([P, W], bf16, tag="ac")
                        bc = abp.tile([P, W], bf16, tag="bc")
                        nc.vector.tensor_copy(ac[:], aT[:])
                        nc.vector.tensor_copy(bc[:], bT[:])

                        def mk(w=w, ci=ci, aT=ac, bT=bc):
                            if w not in colt:
                                # lazy alloc: after the previous window's
                                # drain DMAs, so bank reuse orders cleanly.
                                # 2 banks; a-chain at partition 0, b-chain
                                # at partition 1 (chain i -> bank i%2)
                                colt[w] = [pp.tile([P, 512], f32,
                                                   tag=f"c{i}", bufs=1,
                                                   name=f"col{w}_{i}")
                                           for i in range(2)]
                            ct = colt[w]
                            for i, (src, h) in enumerate(
                                    ((aT, 0), (aT, 1), (bT, 0), (bT, 1))):
                                pr = (i // 2) * 32   # a at 0, b at 32
                                nc.tensor.matmul(
                                    ct[h][pr:pr + 1, 0:512], onesK[:, 0:1],
                                    src[:, bass.ds(h * 512, 512)],
                                    start=(ci == 0), stop=(ci == CI_N - 1))
                            if ci == CI_N - 1:   # chains closed -> drain
                                for i in range(4):
                                    pr = (i // 2) * 32
                                    h = i % 2
                                    nc.vector.tensor_copy(
                                        colstage[0:1, bass.ds(
                                            ((w - 1) * 4 + i) * 512, 512)],
                                        ct[h][pr:pr + 1, 0:512])
                                nc.sync.dma_start(
                                    dcol[0:1, bass.ds((w - 1) * 2048, 2048)],
                                    colstage[0:1, bass.ds((w - 1) * 2048,
                                                          2048)])
                        pend.append(mk)
            for fn in pend:    # flush final tile's colsums (+ drain)
                fn()

            # ── epilogue: raw st tiles out (colsums shipped per window;
            # earlier windows' st columns shipped during the loop) ─────
            for q in range(3):
                for w in range(NW):
                    if (q, w) not in _shipped:
                        nc.sync.dma_start(
                            dout[:, bass.ds(q * ST_W + w * CI_N, CI_N)],
                            st[q][:, bass.ts(w, CI_N)])

    nc.compile()
    return nc


def _get_program():
    if "main" not in _programs:
        _programs["main"] = _build()
    return _programs["main"]


def make_in_maps(x: np.ndarray, y: np.ndarray):
    """Host-precomputed, per-core-rotated bf16 inputs + fp64 aux sums."""
    x = np.ascontiguousarray(np.asarray(x, np.float32))
    y = np.ascontiguousarray(np.asarray(y, np.float32))

    NC = NW * W
    aux = {}
    in_maps = [dict() for _ in range(NCORES)]
    for name, v in (("x", x), ("y", y)):
        vb = v.astype(BF)                      # bf16 of x (matmul stream)
        vT = np.ascontiguousarray(vb.T)        # [128, N] bf16
        vf = vT.astype(np.float32)
        nf32 = (vf * vf).sum(axis=0, dtype=np.float32)      # col norms [N]
        hi = nf32.astype(BF)
        lo = (nf32 - hi.astype(np.float32)).astype(BF)
        vm2 = (-2.0 * v).astype(BF)            # bf16(-2x) (stationary)
        vm2f = vm2.astype(np.float32)
        nb_full = 0.25 * (vm2f * vm2f).sum(axis=1, dtype=np.float32)  # [N]
        aux["sum_n" + name] = nf32.astype(np.float64).sum()
        aux["sum_nb" + name] = nb_full.astype(np.float64).sum()
        aux["s" + name] = vf.astype(np.float64).sum(axis=1)  # [128]
        for c in range(NCORES):
            rl = np.roll(vT, -c * BLK, axis=1)[:, :NC]
            nfr = np.zeros((P, NC), dtype=BF)
            nfr[0] = np.roll(hi, -c * BLK)[:NC]
            nfr[1] = np.roll(lo, -c * BLK)[:NC]
            blkT = np.ascontiguousarray(vm2[c * BLK:(c + 1) * BLK].T)
            nb = np.ascontiguousarray(
                nb_full[c * BLK:(c + 1) * BLK].reshape(CI_N, P).T)
            in_maps[c][name + "T"] = np.ascontiguousarray(rl)
            in_maps[c][name + "blkT"] = blkT
            in_maps[c]["nf" + name] = nfr
            in_maps[c]["nb" + name] = nb.astype(np.float32)

    diagm = (np.eye(P, dtype=np.float32) * MU).astype(BF)
    ew = np.zeros((P, 4 * 512), np.float32)
    for k in range(4):
        for p in range(P):
            ew[p, k * 512 + k * P + p] = MU
    ewb = ew.astype(BF)
    for c in range(NCORES):
        in_maps[c]["diagm"] = diagm
        in_maps[c]["eyewide"] = ewb
    return in_maps, aux


def finalize(results, aux):
    """Combine per-core outputs -> scalar dcor (fp64 host math).

    out cols: st0 (rs_a) 0:40 | st1 (rs_b) 40:80 | st2 (pab) 80:120,
    each [P, w*8+ci]. colsums: per w=1..4: [a_h0|a_h1|b_h0|b_h1]*512.
    Window w of core c covers global column block (c+w)%8; window 4
    contributions are double-covered (pair {c,c+4}) -> weight 1/2.
    Device row sums include the forced diag ~mu (true diag of a is 0).
    """
    n = float(N)
    fw = np.array([1.0, 1.0, 1.0, 1.0, 0.5])
    rs_a = np.zeros(N, np.float64)
    rs_b = np.zeros(N, np.float64)
    pab_w = 0.0        # weighted sum over visited tiles of (a-mu)*b
    pab_w0 = 0.0       # the w=0 (diag-tile) part
    rsw_a = 0.0        # weighted total of row-side sums (same coverage)
    rsw_a0 = 0.0
    rsw_b = 0.0
    rsw_b0 = 0.0
    for c, r in enumerate(results):
        o = np.asarray(r["out"], np.float64)
        cs = np.asarray(r["colsums"], np.float64).reshape(4, 4, 512)
        st0 = o[:, 0:ST_W]
        st1 = o[:, ST_W:2 * ST_W]
        st2 = o[:, 2 * ST_W:3 * ST_W]
        # row-side: rows of block c
        rows = slice(c * BLK, (c + 1) * BLK)
        for w in range(NW):
            pa = st0[:, w * CI_N:(w + 1) * CI_N]   # [P, ci]
            pb = st1[:, w * CI_N:(w + 1) * CI_N]
            rs_a[rows] += fw[w] * pa.T.ravel()
            rs_b[rows] += fw[w] * pb.T.ravel()
            rsw_a += fw[w] * pa.sum()
            rsw_b += fw[w] * pb.sum()
            pab_w += fw[w] * st2[:, w * CI_N:(w + 1) * CI_N].sum()
            if w == 0:
                pab_w0 += st2[:, 0:CI_N].sum()
                rsw_a0 += pa.sum()
                rsw_b0 += pb.sum()
        # mirror-side: col sums of window w -> rows of block (c+w)%8
        for w in range(1, NW):
            g = (c + w) % NCORES
            tgt = slice(g * BLK, (g + 1) * BLK)
            rs_a[tgt] += fw[w] * np.concatenate([cs[w - 1, 0], cs[w - 1, 1]])
            rs_b[tgt] += fw[w] * np.concatenate([cs[w - 1, 2], cs[w - 1, 3]])

    # full-matrix sums: visited set S (weighted) covers each unordered
    # block pair once; (a-mu)*b is symmetric; diag tiles are self-mirrored
    pab = 2.0 * pab_w - pab_w0
    # CONSISTENCY: the global totals sum(a), sum(b) must come from the SAME
    # device accumulators as pab for their systematic errors to cancel in
    # Sab; the bf16-colsum-stitched rs vectors are used only inside the
    # /n-suppressed dot terms below.
    tot_a = 2.0 * rsw_a - rsw_a0    # sum of device-a over the full matrix
    tot_b = 2.0 * rsw_b - rsw_b0    # (incl. the n forced-diag ~mu entries)

    sx = aux["sx"]
    sy = aux["sy"]
    sq_a = n * (aux["sum_nbx"] + aux["sum_nx"]) - 2.0 * np.dot(sx, sx)
    sq_b = n * (aux["sum_nby"] + aux["sum_ny"]) - 2.0 * np.dot(sy, sy)

    sa = rs_a - MU          # true (zero-diag) row sums of a
    sb = rs_b - MU
    sat = sa - n * MU       # row sums of (a - mu)
    sbt = sb - n * MU
    sa_sum = tot_a - n * MU         # sum(sa) from consistent accumulators
    sb_sum = tot_b - n * MU
    Ua = sa_sum - n * n * MU        # sum(sat)
    Ub = sb_sum - n * n * MU
    Sab = pab - MU * (sa_sum - MU * n * n)
    Saa = sq_a - 2.0 * MU * sa_sum + MU2 * n * n
    Sbb = sq_b - 2.0 * MU * sb_sum + MU2 * n * n

    sumAB = Sab - 2.0 * np.dot(sat, sbt) / n + Ua * Ub / n**2
    sumAA = Saa - 2.0 * np.dot(sat, sat) / n + Ua * Ua / n**2
    sumBB = Sbb - 2.0 * np.dot(sbt, sbt) / n + Ub * Ub / n**2

    inv_n2 = 1.0 / (n * n)
    dcov2_xy = sumAB * inv_n2
    dcov2_xx = sumAA * inv_n2
    dcov2_yy = sumBB * inv_n2
    dcor = -np.sqrt(dcov2_xy) / np.sqrt(np.sqrt(dcov2_xx) * np.sqrt(dcov2_yy))
    return np.asarray(dcor, dtype=np.float32)


def run(x, y, mm_mode=None, trace=False, tmpdir=None):
    nc = _get_program()
    in_maps, aux = make_in_maps(x, y)
    res = run_bass_kernel_spmd(nc, in_maps, core_ids=list(range(NCORES)),
                               trace=trace, tmpdir=tmpdir)
    return finalize(res.results, aux), res


def kernel(x, y):
    val, _ = run(x, y)
    return val
